# revision 16
# baseline (speedup 1.0000x reference)
"""Liquid State Machine kernel for Trainium2 (8 NeuronCores).

Strategy: time-sharding with warmup. The recurrence s' = 0.9 s + 0.1 tanh(z)
has fading memory (spectral radius 0.9); starting from zero state ~256 steps
before a core's output window reproduces the true state to fp32 noise
(measured rel err ~2e-7 at 256 warmup steps).

Core 0 computes steps [0, 480) (no warmup needed, true zero initial state).
Core c>=1 computes steps [224c, 224c+480): first 256 steps are warmup, last
224 are its output window. Host gathers: [0,480) from core 0, then 224-step
slices from cores 1..7. Core 7's window ends exactly at t=2048, providing
the final state.

On-chip layout: state sigma (= state/LEAK, folding LEAK into the weights) is
kept feature-major: SBUF [128 part = feat-within-chunk, (kc, slot, b) free].
Per step: z[b, f] accumulated in PSUM via 6 matmuls (rank-1 bias, input
projection with transposed-x stationary, 4 recurrent chunks with sigma
stationary); tanh on ACT (PSUM->SBUF); 4 PE transposes of tanh output;
blend sigma_new = 0.9 sigma + tanh_T on DVE. BN is folded into the weights
host-side. Readout MLP is deferred and batched per L-step chunk with
weights stationary, streaming (slot, batch) columns.
"""

import numpy as np

B, T, IN, LIQ, OUT = 128, 2048, 128, 512, 64
LEAK, EPS = 0.1, 1e-3
NCORES = 8

T_WIN = 480   # steps per core
T_OUT = 224   # output steps for cores 1..7 (core 0 outputs all 480)
L = 16        # steps per on-chip chunk

_built = {}
TRACE = False        # set by test harness to capture an NTFF profile
LAST_RESULTS = None  # BassKernelResults of the most recent run


def _build(t_win: int, l_chunk: int):
    """Build and compile the Bass program for a t_win-step window."""
    import concourse.bacc as bacc
    import concourse.mybir as mybir
    import concourse.tile as tile
    import concourse.masks as masks
    from contextlib import ExitStack

    f32 = mybir.dt.float32
    Tanh = mybir.ActivationFunctionType.Tanh
    Relu = mybir.ActivationFunctionType.Relu
    Ident = mybir.ActivationFunctionType.Identity

    nch = t_win // l_chunk
    assert nch * l_chunk == t_win

    nc = bacc.Bacc("TRN2", target_bir_lowering=False, debug=False)

    # DRAM I/O (weight layouts are pre-transposed host-side; all DMAs are
    # plain 2D copies).  x arrives feature-major [k, t, b]; y leaves
    # feature-major [o, t*128+b] and the host transposes it back.
    x_d = nc.dram_tensor("x", [IN, t_win, B], f32, kind="ExternalInput")
    whk_d = nc.dram_tensor("whk", [128, 4 * LIQ], f32, kind="ExternalInput")
    wxf_d = nc.dram_tensor("wxf", [IN, LIQ], f32, kind="ExternalInput")
    bf_d = nc.dram_tensor("bf", [1, LIQ], f32, kind="ExternalInput")
    w1t_d = nc.dram_tensor("w1t", [128, 8 * 128], f32, kind="ExternalInput")
    b1_d = nc.dram_tensor("b1c", [128, 2], f32, kind="ExternalInput")
    w2t_d = nc.dram_tensor("w2t", [128, 2 * 128], f32, kind="ExternalInput")
    b2_d = nc.dram_tensor("b2c", [128, 1], f32, kind="ExternalInput")
    w3_d = nc.dram_tensor("w3", [128, OUT], f32, kind="ExternalInput")
    b3_d = nc.dram_tensor("b3c", [OUT, 1], f32, kind="ExternalInput")
    y_d = nc.dram_tensor("y", [OUT, t_win * B], f32, kind="ExternalOutput")
    sf_d = nc.dram_tensor("sf", [128, 4 * 128], f32, kind="ExternalOutput")

    with tile.TileContext(nc) as tc, ExitStack() as ctx:
        const_pool = ctx.enter_context(tc.tile_pool(name="consts", bufs=1))
        sig_pool = ctx.enter_context(tc.tile_pool(name="sigma", bufs=2))
        xt_pool = ctx.enter_context(tc.tile_pool(name="xtchunk", bufs=2))
        th_pool = ctx.enter_context(tc.tile_pool(name="th", bufs=3))
        h1_pool = ctx.enter_context(tc.tile_pool(name="h1", bufs=1))
        h2_pool = ctx.enter_context(tc.tile_pool(name="h2", bufs=1))
        yo_pool = ctx.enter_context(tc.tile_pool(name="yout", bufs=2))
        pz_pool = ctx.enter_context(tc.tile_pool(name="pz", bufs=2, space="PSUM"))
        pt_pool = ctx.enter_context(tc.tile_pool(name="pt", bufs=2, space="PSUM"))
        ph_pool = ctx.enter_context(tc.tile_pool(name="ph", bufs=3, space="PSUM"))
        py_pool = ctx.enter_context(tc.tile_pool(name="py", bufs=1, space="PSUM"))

        def load_const(name, shape, dram):
            t = const_pool.tile(shape, f32, tag=name)
            nc.sync.dma_start(t[:], dram[:])
            return t

        whk = load_const("whk", [128, 4 * LIQ], whk_d)
        wxf = load_const("wxf", [IN, LIQ], wxf_d)
        bf = load_const("bf", [1, LIQ], bf_d)
        w1t = load_const("w1t", [128, 8 * 128], w1t_d)
        b1c = load_const("b1c", [128, 2], b1_d)
        w2t = load_const("w2t", [128, 2 * 128], w2t_d)
        b2c = load_const("b2c", [128, 1], b2_d)
        w3 = load_const("w3", [128, OUT], w3_d)
        b3c = load_const("b3c", [OUT, 1], b3_d)

        ident = const_pool.tile([128, 128], f32, tag="ident")
        masks.make_identity(nc, ident[:])
        ones = const_pool.tile([1, 128], f32, tag="ones")
        nc.vector.memset(ones[:], 1.0)

        # sigma store: [128 part=feat, (kc, slot, b)]; slot 0 = carry-in state
        nslot = l_chunk + 1

        def sigv(tile_):
            return tile_[:].rearrange(
                "p (kc s b) -> p kc s b", kc=4, s=nslot, b=128
            )

        sig_prev = None
        for c in range(nch):
            t0 = c * l_chunk
            sig = sig_pool.tile([128, 4 * nslot * 128], f32, tag="sig")
            if c == 0:
                nc.vector.memset(sigv(sig)[:, :, 0, :], 0.0)
            else:
                nc.vector.tensor_copy(
                    sigv(sig)[:, :, 0, :], sigv(sig_prev)[:, :, l_chunk, :]
                )

            # load x chunk, already feature-major from the host: [128 k, (t b)]
            xt = xt_pool.tile([128, l_chunk * B], f32, tag="xt")
            nc.sync.dma_start(
                xt[:], x_d[:, t0 : t0 + l_chunk, :].rearrange("k t b -> k (t b)")
            )

            # recurrence
            for li in range(l_chunk):
                z = pz_pool.tile([128, LIQ], f32, tag="z")
                nc.tensor.matmul(z[:], ones[:], bf[:], start=True, stop=False)
                nc.tensor.matmul(
                    z[:],
                    xt[:, li * 128 : (li + 1) * 128],
                    wxf[:],
                    start=False,
                    stop=False,
                )
                for kc in range(4):
                    nc.tensor.matmul(
                        z[:],
                        sigv(sig)[:, kc, li, :],
                        whk[:, kc * LIQ : (kc + 1) * LIQ],
                        start=False,
                        stop=(kc == 3),
                    )
                th = th_pool.tile([128, LIQ], f32, tag="th")
                nc.scalar.activation(th[:], z[:], Tanh)
                tt = pt_pool.tile([128, LIQ], f32, tag="tt")
                for kc in range(4):
                    nc.tensor.transpose(
                        tt[:, kc * 128 : (kc + 1) * 128],
                        th[:, kc * 128 : (kc + 1) * 128],
                        ident[:],
                    )
                nc.vector.tensor_scalar_mul(
                    sigv(sig)[:, :, li + 1, :], sigv(sig)[:, :, li, :], 1.0 - LEAK
                )
                nc.vector.tensor_add(
                    sigv(sig)[:, :, li + 1, :], sigv(sig)[:, :, li + 1, :], tt[:]
                )

            # readout over slots 1..l_chunk  (states after update at each step)
            nfree = l_chunk * 128
            nsl512 = nfree // 512
            h1 = h1_pool.tile([128, 2 * nfree], f32, tag="h1")
            for mc in range(2):
                for n in range(nsl512):
                    ph = ph_pool.tile([128, 512], f32, tag="ph")
                    for kc in range(4):
                        nc.tensor.matmul(
                            ph[:],
                            w1t[:, (kc * 2 + mc) * 128 : (kc * 2 + mc + 1) * 128],
                            sigv(sig)[:, kc, 1 + n * 4 : 1 + (n + 1) * 4, :],
                            start=(kc == 0),
                            stop=(kc == 3),
                        )
                    nc.scalar.activation(
                        h1[:, mc * nfree + n * 512 : mc * nfree + (n + 1) * 512],
                        ph[:],
                        Relu,
                        bias=b1c[:, mc : mc + 1],
                    )
            h2 = h2_pool.tile([128, nfree], f32, tag="h2")
            for n in range(nsl512):
                ph = ph_pool.tile([128, 512], f32, tag="ph")
                for kc in range(2):
                    nc.tensor.matmul(
                        ph[:],
                        w2t[:, kc * 128 : (kc + 1) * 128],
                        h1[:, kc * nfree + n * 512 : kc * nfree + (n + 1) * 512],
                        start=(kc == 0),
                        stop=(kc == 1),
                    )
                nc.scalar.activation(
                    h2[:, n * 512 : (n + 1) * 512], ph[:], Relu, bias=b2c[:, 0:1]
                )
            yo = yo_pool.tile([OUT, l_chunk * B], f32, tag="yo")
            for n in range(nsl512):
                py = py_pool.tile([OUT, 512], f32, tag="py")
                nc.tensor.matmul(
                    py[:], w3[:], h2[:, n * 512 : (n + 1) * 512], start=True, stop=True
                )
                nc.scalar.activation(
                    yo[:, n * 512 : (n + 1) * 512],
                    py[:],
                    Ident,
                    bias=b3c[:, 0:1],
                )
            nc.sync.dma_start(y_d[:, t0 * B : (t0 + l_chunk) * B], yo[:])
            if c == nch - 1:
                nc.sync.dma_start(sf_d[:], sigv(sig)[:, :, l_chunk, :])
            sig_prev = sig

    nc.compile()
    return nc


def _prep_weights(W, b, gamma, beta, mov_mean, mov_var, W1, b1, W2, b2, W3, b3):
    """Fold BN into the liquid weights and LEAK into the state scaling.

    sigma = state / LEAK;  sigma' = 0.9 sigma + tanh(zbn)
    zbn = x @ WxF + sigma @ WhK + bF;  h1 = relu(sigma @ (LEAK*W1) + b1)
    """
    W = np.asarray(W, np.float64)
    scale = np.asarray(gamma, np.float64) / np.sqrt(
        np.asarray(mov_var, np.float64) + EPS
    )
    shift = np.asarray(beta, np.float64) - np.asarray(mov_mean, np.float64) * scale
    Wx = (W[:IN] * scale).astype(np.float32)          # [IN, LIQ]
    Wh = (W[IN:] * scale * LEAK).astype(np.float32)   # [LIQ, LIQ]
    bF = (np.asarray(b, np.float64) * scale + shift).astype(np.float32)
    W1K = (np.asarray(W1, np.float64) * LEAK).astype(np.float32)  # [LIQ, 256]
    return {
        "wxf": np.ascontiguousarray(Wx),
        # whk[p, kc*512+n] = Wh[kc*128+p, n]
        "whk": np.ascontiguousarray(
            Wh.reshape(4, 128, LIQ).transpose(1, 0, 2).reshape(128, 4 * LIQ)
        ),
        "bf": bF.reshape(1, LIQ),
        # w1t[p, (kc*2+mc)*128+n] = W1K[kc*128+p, mc*128+n]
        "w1t": np.ascontiguousarray(
            W1K.reshape(4, 128, 2, 128).transpose(1, 0, 2, 3).reshape(128, 8 * 128)
        ),
        "b1c": np.ascontiguousarray(np.asarray(b1, np.float32).reshape(2, 128).T),
        # w2t[p, kc*128+n] = W2[kc*128+p, n]
        "w2t": np.ascontiguousarray(
            np.asarray(W2, np.float32).reshape(2, 128, 128).transpose(1, 0, 2)
            .reshape(128, 2 * 128)
        ),
        "b2c": np.asarray(b2, np.float32).reshape(128, 1),
        "w3": np.ascontiguousarray(np.asarray(W3, np.float32)),
        "b3c": np.asarray(b3, np.float32).reshape(OUT, 1),
    }


def kernel(inputs, W, b, gamma, beta, mov_mean, mov_var, W1, b1, W2, b2, W3, b3):
    from concourse.bass_utils import run_bass_kernel_spmd

    x = np.asarray(inputs, np.float32)  # [B, T, IN]
    wts = _prep_weights(W, b, gamma, beta, mov_mean, mov_var, W1, b1, W2, b2, W3, b3)

    key = (T_WIN, L)
    if key not in _built:
        _built[key] = _build(T_WIN, L)
    nc = _built[key]

    xT = np.ascontiguousarray(x.transpose(2, 1, 0))  # [k, t, b]
    starts = [0] + [224 * c for c in range(1, NCORES)]
    in_maps = []
    for c in range(NCORES):
        m = dict(wts)
        m["x"] = np.ascontiguousarray(xT[:, starts[c] : starts[c] + T_WIN, :])
        in_maps.append(m)

    global LAST_RESULTS
    LAST_RESULTS = run_bass_kernel_spmd(
        nc, in_maps, list(range(NCORES)), trace=TRACE
    )
    res = LAST_RESULTS.results

    out = np.empty((B, T, OUT), np.float32)
    yw = res[0]["y"].reshape(OUT, T_WIN, B)
    out[:, 0:T_WIN, :] = yw.transpose(2, 1, 0)
    for c in range(1, NCORES):
        lo = T_WIN + T_OUT * (c - 1)
        yw = res[c]["y"].reshape(OUT, T_WIN, B)[:, T_WIN - T_OUT :, :]
        out[:, lo : lo + T_OUT, :] = yw.transpose(2, 1, 0)

    # final state from core 7: sf[p, kc*128+b] = sigma[kc*128+p, b]
    sf = res[NCORES - 1]["sf"].reshape(128, 4, 128)
    final_state = (LEAK * np.transpose(sf, (2, 1, 0)).reshape(B, LIQ)).astype(
        np.float32
    )
    return out, final_state
